# revision 17
# baseline (speedup 1.0000x reference)
"""Trainium2 Bass kernel for cross-attention with per-head structured mask.

Reference computation (B=4, N=1024, DIM=1024, H=16, D=64):
    q = x1 @ Wq;  k, v = split(x2 @ Wkv)
    dots = q k^T * D^-0.5 + spd
    attn = softmax(dots) * (head_keep * H / n_kept)   # whole heads dropped
    out  = (attn @ v) @ Wo + bo

Sharding: dropped heads contribute exactly zero, so only kept heads are
computed. Work unit = (batch b, kept-head group g): 8 cores = 4 batches x 2
head groups. Each core computes a partial out[b] (its heads' contribution
through Wo); host sums the two partials per batch and adds the bias.

Device layout (per core, H_c heads, all fp32):
    QT[hd, n], KT[hd, m] via PE (contraction over DIM, inputs pre-transposed
    on host).  V held as [m, h, 128] blocks: per head, 64 cols of V plus 64
    cols of ones (parity-swapped), so ctx_psum = V_aug^T @ exp(scores^T)
    carries both the context rows AND the softmax denominator rows in one
    accumulation, with the rows landing at the partition base the final
    ctxT layout needs (no cross-partition moves on DVE).
"""

import math
import os

import numpy as np

B, N, DIM = 4, 1024, 1024
HEADS, DIM_HEAD = 16, 64
INNER = HEADS * DIM_HEAD
SCALE = DIM_HEAD ** -0.5
NCORES = 8
KT = DIM // 128      # 8 contraction tiles
NB = N // 512        # 2 column blocks
MT = N // 128        # 8 key tiles

_cache: dict = {}


def _build(H_c: int, keep_scale: float, use_f32r: bool = True, debug_taps: bool = False):
    """Build + compile the per-core Bass program for H_c heads (H_c even)."""
    import concourse.bass as bass
    import concourse.mybir as mybir
    import concourse.tile as tile
    from concourse import bacc

    dt = mybir.dt
    f32 = dt.float32
    HB = H_c // 2
    HD = H_c * DIM_HEAD
    assert H_c % 2 == 0 and HD <= 512

    mmdt = dt.float32r if use_f32r else dt.float32

    def mm(ap):
        return ap

    nc = bacc.Bacc("TRN2", target_bir_lowering=False)

    xq = nc.dram_tensor("xq", [KT, 128, N], mmdt, kind="ExternalInput")   # x1[b].T tiled
    xk = nc.dram_tensor("xk", [KT, 128, N], mmdt, kind="ExternalInput")   # x2[b].T tiled
    wq = nc.dram_tensor("wq", [128, KT, HD], mmdt, kind="ExternalInput")  # sbuf layout
    wk = nc.dram_tensor("wk", [128, KT, HD], mmdt, kind="ExternalInput")
    wv = nc.dram_tensor("wv", [128, KT, HD], mmdt, kind="ExternalInput")
    wo = nc.dram_tensor("wo", [128, HD // 128, DIM], mmdt, kind="ExternalInput")
    spd = nc.dram_tensor("spd", [H_c, N, N], f32, kind="ExternalInput")  # spd[b,h].T
    out = nc.dram_tensor("out", [N, DIM], f32, kind="ExternalOutput")
    if debug_taps:
        dbg_qt = nc.dram_tensor("dbg_qt", [128, HB, N], f32, kind="ExternalOutput")
        dbg_kt = nc.dram_tensor("dbg_kt", [128, HB, N], f32, kind="ExternalOutput")
        dbg_v = nc.dram_tensor("dbg_v", [128, MT, H_c * 128], f32, kind="ExternalOutput")
        dbg_ct = nc.dram_tensor("dbg_ct", [128, HB, N], f32, kind="ExternalOutput")
        dbg_ctxp = nc.dram_tensor("dbg_ctxp", [H_c, NB, 128, 512], f32, kind="ExternalOutput")
        dbg_rbs = nc.dram_tensor("dbg_rbs", [H_c, NB, 128, 512], f32, kind="ExternalOutput")
        dbg_e = nc.dram_tensor("dbg_e", [H_c, NB, 128, 512], f32, kind="ExternalOutput")
        dbg_rr = nc.dram_tensor("dbg_rr", [H_c, NB, 2, 512], f32, kind="ExternalOutput")

    Exp = mybir.ActivationFunctionType.Exp
    mult = mybir.AluOpType.mult

    with tile.TileContext(nc) as tc:
        with (
            tc.tile_pool(name="w", bufs=1) as wpool,
            tc.tile_pool(name="big", bufs=1) as big,
            tc.tile_pool(name="xs", bufs=4) as xs,
            tc.tile_pool(name="spdp", bufs=6) as spdp,
            tc.tile_pool(name="work", bufs=4) as work,
            tc.tile_pool(name="psA", bufs=4, space="PSUM") as psA,
            tc.tile_pool(name="psS", bufs=2, space="PSUM") as psS,
            tc.tile_pool(name="psC", bufs=2, space="PSUM") as psC,
        ):
            wq_sb = wpool.tile([128, KT, HD], mmdt, tag="wq")
            wk_sb = wpool.tile([128, KT, HD], mmdt, tag="wk")
            wv_sb = wpool.tile([128, KT, HD], mmdt, tag="wv")
            wo_sb = wpool.tile([128, HD // 128, DIM], mmdt, tag="wo")
            for dst, src in ((wq_sb, wq), (wk_sb, wk), (wv_sb, wv), (wo_sb, wo)):
                nc.sync.dma_start(dst[:], src[:])

            qt_sb = big.tile([128, HB, N], mmdt, tag="qt")
            kt_sb = big.tile([128, HB, N], mmdt, tag="kt")
            v_sb = big.tile([128, MT, H_c * 128], mmdt, tag="v")
            ct_sb = big.tile([128, HB, N], mmdt, tag="ct")

            ones_sb = wpool.tile([128, 64], f32, tag="ones")
            nc.gpsimd.memset(ones_sb[:], 1.0)

            # ones columns of the augmented V blocks (parity-swapped per head)
            for h in range(H_c):
                c0 = h * 128 + (64 if h % 2 == 0 else 0)
                for m in range(MT):
                    nc.scalar.copy(v_sb[:, m, c0:c0 + 64], ones_sb[:])

            # ---- Phase A1: QT[hd, n] and KT[hd, m] projections ----
            for src, w_sb, dst in ((xq, wq_sb, qt_sb), (xk, wk_sb, kt_sb)):
                for n0 in range(NB):
                    ps = [psA.tile([128, 512], f32, tag="acc", name=f"psqk{n0}_{i}") for i in range(HB)]
                    for k in range(KT):
                        xt = xs.tile([128, 512], mmdt, tag="x")
                        nc.sync.dma_start(xt[:], src[k, :, n0 * 512:(n0 + 1) * 512])
                        for hb in range(HB):
                            nc.tensor.matmul(
                                ps[hb][:],
                                mm(w_sb[:, k, hb * 128:(hb + 1) * 128]),
                                mm(xt[:]),
                                start=(k == 0),
                                stop=(k == KT - 1),
                            )
                    for hb in range(HB):
                        nc.scalar.copy(dst[:, hb, n0 * 512:(n0 + 1) * 512], ps[hb][:])

            # ---- Phase A2: V[m, hd] projection (normal layout) ----
            for m0 in range(NB):
                ps = [psA.tile([128, HD], f32, tag="acc", name=f"psv{m0}_{i}") for i in range(4)]
                for k in range(KT):
                    xt = xs.tile([128, 512], mmdt, tag="x")
                    nc.sync.dma_start(xt[:], xk[k, :, m0 * 512:(m0 + 1) * 512])
                    for mi in range(4):
                        nc.tensor.matmul(
                            ps[mi][:],
                            mm(xt[:, mi * 128:(mi + 1) * 128]),
                            mm(wv_sb[:, k, :]),
                            start=(k == 0),
                            stop=(k == KT - 1),
                        )
                for mi in range(4):
                    m = m0 * 4 + mi
                    pv = ps[mi][:].rearrange("p (hb two d) -> p hb two d", two=2, d=64)
                    vv = v_sb[:, m, :].rearrange("p (hb x) -> p hb x", hb=HB)
                    # even heads -> value cols 0:64 of their block; odd -> 192:256
                    nc.scalar.copy(vv[:, :, 0:64], pv[:, :, 0, :])
                    nc.scalar.copy(vv[:, :, 192:256], pv[:, :, 1, :])

            # ---- Phase B: per (head, n-block) attention ----
            for h in range(H_c):
                hb, hp = divmod(h, 2)
                vb = hp * 64          # partition base of ctx values
                sb_ = 64 - vb         # partition base of sumexp rows
                for n0 in range(NB):
                    n_sl = slice(n0 * 512, (n0 + 1) * 512)
                    ctx = psC.tile([128, 512], f32, tag="ctx")
                    for m in range(MT):
                        sc = psS.tile([128, 512], f32, tag="sc")
                        nc.tensor.matmul(
                            sc[:],
                            mm(kt_sb[vb:vb + 64, hb, m * 128:(m + 1) * 128]),
                            mm(qt_sb[vb:vb + 64, hb, n_sl]),
                        )
                        spdt = spdp.tile([128, 512], f32, tag="spd")
                        nc.sync.dma_start(
                            spdt[:], spd[h, m * 128:(m + 1) * 128, n_sl]
                        )
                        s = work.tile([128, 512], f32, tag="s")
                        nc.vector.tensor_add(s[:], sc[:], spdt[:])
                        e = work.tile([128, 512], mmdt, tag="e")
                        nc.scalar.activation(e[:], s[:], Exp)
                        if debug_taps and m == 0:
                            nc.sync.dma_start(dbg_e[h, n0], e[:].bitcast(f32))
                        nc.tensor.matmul(
                            ctx[:],
                            mm(v_sb[:, m, h * 128:(h + 1) * 128]),
                            mm(e[:]),
                            start=(m == 0),
                            stop=(m == MT - 1),
                        )
                    # normalize: ctxT = ctx_vals * keep_scale / sumexp
                    if debug_taps:
                        dbgt = work.tile([128, 512], f32, tag="dbgt", name=f"dt{h}_{n0}")
                        nc.scalar.copy(dbgt[:], ctx[:])
                        nc.sync.dma_start(dbg_ctxp[h, n0], dbgt[:])
                    # custom-DVE ops only behave at partition base 0 on HW:
                    # for even heads the sumexp row sits at partition 64, so
                    # ACT-copy it to SBUF and DMA it down to partition 0.
                    rr = work.tile([128, 512], f32, tag="rr")
                    if sb_ == 0:
                        recip_in = ctx[0:1, :]
                    else:
                        sscr = work.tile([128, 512], f32, tag="sscr")
                        nc.scalar.copy(sscr[sb_:sb_ + 1, :], ctx[sb_:sb_ + 1, :])
                        sm0 = work.tile([128, 512], f32, tag="sm0")
                        nc.sync.dma_start(sm0[0:1, :], sscr[sb_:sb_ + 1, :])
                        recip_in = sm0[0:1, :]
                    nc.vector.reciprocal_approx_fast(rr[0:1, :], recip_in)
                    rr_src = rr[0:1, :]
                    # broadcast 1/sumexp across the 64 value partitions via a
                    # K=1 ones-matmul (PE can cross partitions; DVE cannot)
                    rb = psS.tile([128, 512], f32, tag="sc", name=f"rb{h}_{n0}")
                    if debug_taps:
                        nc.sync.dma_start(dbg_rr[h, n0, 0:1, :], rr[sb_:sb_ + 1, :])
                        nc.sync.dma_start(dbg_rr[h, n0, 1:2, :], rr_src)
                    nc.tensor.matmul(
                        rb[vb:vb + 64, :],
                        ones_sb[0:1, :],
                        rr_src,
                    )
                    rbs = work.tile([128, 512], f32, tag="rbs")
                    nc.scalar.copy(rbs[vb:vb + 64, :], rb[vb:vb + 64, :])
                    if debug_taps:
                        nc.sync.dma_start(dbg_rbs[h, n0], rbs[:])
                    nc.vector.scalar_tensor_tensor(
                        out=ct_sb[vb:vb + 64, hb, n_sl],
                        in0=ctx[vb:vb + 64, :],
                        scalar=float(keep_scale),
                        in1=rbs[vb:vb + 64, :],
                        op0=mult,
                        op1=mult,
                    )

            if debug_taps:
                for t_sb, t_dr in ((qt_sb, dbg_qt), (kt_sb, dbg_kt),
                                   (v_sb, dbg_v), (ct_sb, dbg_ct)):
                    nc.sync.dma_start(t_dr[:], t_sb[:].bitcast(f32))

            # ---- Phase C: out[n, dim] = ctxT^T @ Wo ----
            for nt in range(MT):
                for d0 in range(NB):
                    po = psA.tile([128, 512], f32, tag="acc")
                    for kk in range(HD // 128):
                        nc.tensor.matmul(
                            po[:],
                            mm(ct_sb[:, kk, nt * 128:(nt + 1) * 128]),
                            mm(wo_sb[:, kk, d0 * 512:(d0 + 1) * 512]),
                            start=(kk == 0),
                            stop=(kk == HD // 128 - 1),
                        )
                    ot = work.tile([128, 512], f32, tag="o")
                    nc.scalar.copy(ot[:], po[:])
                    nc.sync.dma_start(
                        out[nt * 128:(nt + 1) * 128, d0 * 512:(d0 + 1) * 512], ot[:]
                    )

    nc.finalize()
    return nc


def _get_nc(H_c: int, n_kept: int, use_f32r: bool = True):
    key = (H_c, n_kept, use_f32r)
    if key not in _cache:
        _cache[key] = _build(H_c, HEADS / n_kept, use_f32r)
    return _cache[key]


def _prep_inputs(x1, x2, spd, head_keep, Wq, Wkv, Wo):
    """Slice/transpose/pad host-side into per-core input maps."""
    kept = [int(i) for i in np.nonzero(head_keep)[0]]
    n_kept = len(kept)
    H_c = (n_kept + 1) // 2
    if H_c % 2:
        H_c += 1
    groups = [kept[:H_c], kept[H_c:]]

    Wk_full, Wv_full = Wkv[:, :INNER], Wkv[:, INNER:]

    in_maps = []
    for b in range(B):
        xqT = np.ascontiguousarray(x1[b].T).reshape(KT, 128, N)
        xkT = np.ascontiguousarray(x2[b].T).reshape(KT, 128, N)
        for g in range(2):
            heads = groups[g]
            HD = H_c * DIM_HEAD
            wq_c = np.zeros((DIM, HD), np.float32)
            wk_c = np.zeros((DIM, HD), np.float32)
            wv_c = np.zeros((DIM, HD), np.float32)
            wo_c = np.zeros((HD, DIM), np.float32)
            spd_c = np.zeros((H_c, N, N), np.float32)
            for i, h in enumerate(heads):
                sl = slice(i * DIM_HEAD, (i + 1) * DIM_HEAD)
                hs = slice(h * DIM_HEAD, (h + 1) * DIM_HEAD)
                wq_c[:, sl] = Wq[:, hs] * SCALE
                wk_c[:, sl] = Wk_full[:, hs]
                wv_c[:, sl] = Wv_full[:, hs]
                wo_c[sl, :] = Wo[hs, :]
                spd_c[i] = spd[b, h].T
            in_maps.append({
                "xq": xqT,
                "xk": xkT,
                "wq": np.ascontiguousarray(wq_c.reshape(KT, 128, HD).transpose(1, 0, 2)),
                "wk": np.ascontiguousarray(wk_c.reshape(KT, 128, HD).transpose(1, 0, 2)),
                "wv": np.ascontiguousarray(wv_c.reshape(KT, 128, HD).transpose(1, 0, 2)),
                "wo": np.ascontiguousarray(
                    wo_c.reshape(HD // 128, 128, DIM).transpose(1, 0, 2)
                ),
                "spd": spd_c,
            })
    return in_maps, n_kept, H_c


def _run(nc, in_maps, trace=False, tmpdir=None):
    from concourse.bass_utils import run_bass_kernel_spmd

    return run_bass_kernel_spmd(
        nc, in_maps, core_ids=list(range(NCORES)), trace=trace, tmpdir=tmpdir
    )


def kernel(x1, x2, spd, head_keep, Wq, Wkv, Wo, bo, _trace=False, _tmpdir=None):
    x1 = np.asarray(x1, np.float32)
    x2 = np.asarray(x2, np.float32)
    spd = np.asarray(spd, np.float32)
    head_keep = np.asarray(head_keep)
    n_kept = int(head_keep.astype(np.int64).sum())
    if n_kept == 0:
        # reference: 16/0 = inf, 0*inf = nan everywhere
        return np.full((B, N, DIM), np.nan, np.float32)

    in_maps, n_kept, H_c = _prep_inputs(x1, x2, spd, head_keep, Wq, Wkv, Wo)
    use_f32r = os.environ.get("KERNEL_F32R", "1") == "1"
    nc = _get_nc(H_c, n_kept, use_f32r)
    res = _run(nc, in_maps, trace=_trace, tmpdir=_tmpdir)

    out = np.empty((B, N, DIM), np.float32)
    bo32 = np.asarray(bo, np.float32)
    for b in range(B):
        out[b] = res.results[2 * b]["out"] + res.results[2 * b + 1]["out"] + bo32
    kernel._last_results = res
    return out


# revision 18
# speedup vs baseline: 1.2497x; 1.2497x over previous
"""Trainium2 Bass kernel for cross-attention with per-head structured mask.

Reference computation (B=4, N=1024, DIM=1024, H=16, D=64):
    q = x1 @ Wq;  k, v = split(x2 @ Wkv)
    dots = q k^T * D^-0.5 + spd
    attn = softmax(dots) * (head_keep * H / n_kept)   # whole heads dropped
    out  = (attn @ v) @ Wo + bo

Sharding: dropped heads contribute exactly zero, so only kept heads are
computed. Work unit = (batch b, kept-head group g): 8 cores = 4 batches x 2
head groups. Each core computes a partial out[b] (its heads' contribution
through Wo); host sums the two partials per batch and adds the bias.

Device layout (per core, H_c heads):
    QT[hd, n], KT[hd, m] via PE (contraction over DIM, inputs pre-transposed
    on host).  V held as [m, h, 128] blocks: per head, 64 cols of V plus 64
    cols of ones (parity-swapped), so ctx_psum = V_aug^T @ exp(scores^T)
    carries both the context rows AND the softmax denominator rows in one
    accumulation, landing at the partition base the final ctxT layout needs.

HW quirks baked in (found empirically):
  - custom-DVE ops (reciprocal_approx_fast) and K=1 matmul operands only
    behave at partition base 0 -> shuttle rows down via tiny DMAs.
  - fp32/fp32r matmuls run at ~2-4 cyc/row; bf16 runs at 1 cyc/row, so
    matmul operands default to bf16 (PSUM accumulation stays fp32).
"""

import os

import numpy as np

B, N, DIM = 4, 1024, 1024
HEADS, DIM_HEAD = 16, 64
INNER = HEADS * DIM_HEAD
SCALE = DIM_HEAD ** -0.5
NCORES = 8
KT = DIM // 128      # 8 contraction tiles
NB = N // 512        # 2 column blocks
MT = N // 128        # 8 key tiles

_cache: dict = {}


def _build(H_c: int, keep_scale: float, mode: str = "bf16", debug_taps: bool = False):
    """Build + compile the per-core Bass program for H_c heads (H_c even)."""
    import concourse.mybir as mybir
    import concourse.tile as tile
    from concourse import bacc

    dt = mybir.dt
    f32 = dt.float32
    HB = H_c // 2
    HD = H_c * DIM_HEAD
    assert H_c % 2 == 0 and HD <= 512

    mmdt = {"bf16": dt.bfloat16, "f32r": dt.float32r, "f32": f32}[mode]

    nc = bacc.Bacc("TRN2", target_bir_lowering=False)

    xq = nc.dram_tensor("xq", [KT, 128, N], mmdt, kind="ExternalInput")   # x1[b].T tiled
    xk = nc.dram_tensor("xk", [KT, 128, N], mmdt, kind="ExternalInput")   # x2[b].T tiled
    wq = nc.dram_tensor("wq", [128, KT, HD], mmdt, kind="ExternalInput")  # sbuf layout
    wk = nc.dram_tensor("wk", [128, KT, HD], mmdt, kind="ExternalInput")
    wv = nc.dram_tensor("wv", [128, KT, HD], mmdt, kind="ExternalInput")
    wo = nc.dram_tensor("wo", [128, HD // 128, DIM], mmdt, kind="ExternalInput")
    spd = nc.dram_tensor("spd", [H_c, N, N], mmdt, kind="ExternalInput")  # spd[b,h].T
    out = nc.dram_tensor("out", [N, DIM], f32, kind="ExternalOutput")

    Exp = mybir.ActivationFunctionType.Exp
    mult = mybir.AluOpType.mult

    with tile.TileContext(nc) as tc:
        with (
            tc.tile_pool(name="w", bufs=1) as wpool,
            tc.tile_pool(name="big", bufs=1) as big,
            tc.tile_pool(name="xs", bufs=4) as xs,
            tc.tile_pool(name="spdp", bufs=6) as spdp,
            tc.tile_pool(name="work", bufs=4) as work,
            tc.tile_pool(name="psA", bufs=4, space="PSUM") as psA,
            tc.tile_pool(name="psS", bufs=2, space="PSUM") as psS,
            tc.tile_pool(name="psC", bufs=2, space="PSUM") as psC,
        ):
            wq_sb = wpool.tile([128, KT, HD], mmdt, tag="wq")
            wk_sb = wpool.tile([128, KT, HD], mmdt, tag="wk")
            wv_sb = wpool.tile([128, KT, HD], mmdt, tag="wv")
            wo_sb = wpool.tile([128, HD // 128, DIM], mmdt, tag="wo")
            for dst, src in ((wq_sb, wq), (wk_sb, wk), (wv_sb, wv), (wo_sb, wo)):
                nc.sync.dma_start(dst[:], src[:])

            ones_sb = wpool.tile([128, 64], f32, tag="ones")
            nc.gpsimd.memset(ones_sb[:], 1.0)

            qt_sb = big.tile([128, HB, N], mmdt, tag="qt")
            kt_sb = big.tile([128, HB, N], mmdt, tag="kt")
            v_sb = big.tile([128, MT, H_c * 128], mmdt, tag="v")
            ct_sb = big.tile([128, HB, N], mmdt, tag="ct")

            # ones columns of the augmented V blocks (parity-swapped per head)
            for h in range(H_c):
                c0 = h * 128 + (64 if h % 2 == 0 else 0)
                for m in range(MT):
                    nc.scalar.copy(v_sb[:, m, c0:c0 + 64], ones_sb[:])

            # ---- Phase A1: QT[hd, n] and KT[hd, m] projections ----
            for src, w_sb, dst in ((xq, wq_sb, qt_sb), (xk, wk_sb, kt_sb)):
                for n0 in range(NB):
                    ps = [psA.tile([128, 512], f32, tag="acc",
                                   name=f"psqk{n0}_{i}") for i in range(HB)]
                    for k in range(KT):
                        xt = xs.tile([128, 512], mmdt, tag="x")
                        nc.sync.dma_start(xt[:], src[k, :, n0 * 512:(n0 + 1) * 512])
                        for hb in range(HB):
                            nc.tensor.matmul(
                                ps[hb][:],
                                w_sb[:, k, hb * 128:(hb + 1) * 128],
                                xt[:],
                                start=(k == 0),
                                stop=(k == KT - 1),
                            )
                    for hb in range(HB):
                        nc.scalar.copy(dst[:, hb, n0 * 512:(n0 + 1) * 512], ps[hb][:])

            # ---- Phase A2: V[m, hd] projection (normal layout) ----
            for m0 in range(NB):
                ps = [psA.tile([128, HD], f32, tag="acc",
                               name=f"psv{m0}_{i}") for i in range(4)]
                for k in range(KT):
                    xt = xs.tile([128, 512], mmdt, tag="x")
                    nc.sync.dma_start(xt[:], xk[k, :, m0 * 512:(m0 + 1) * 512])
                    for mi in range(4):
                        nc.tensor.matmul(
                            ps[mi][:],
                            xt[:, mi * 128:(mi + 1) * 128],
                            wv_sb[:, k, :],
                            start=(k == 0),
                            stop=(k == KT - 1),
                        )
                for mi in range(4):
                    m = m0 * 4 + mi
                    pv = ps[mi][:].rearrange("p (hb two d) -> p hb two d", two=2, d=64)
                    vv = v_sb[:, m, :].rearrange("p (hb x) -> p hb x", hb=HB)
                    # even heads -> value cols 0:64 of their block; odd -> 192:256
                    nc.scalar.copy(vv[:, :, 0:64], pv[:, :, 0, :])
                    nc.scalar.copy(vv[:, :, 192:256], pv[:, :, 1, :])

            # ---- Phase B (attention) + C (out proj), interleaved per n-block ----
            for n0 in range(NB):
                n_sl = slice(n0 * 512, (n0 + 1) * 512)
                for h in range(H_c):
                    hb, hp = divmod(h, 2)
                    vb = hp * 64          # partition base of ctx values
                    sb_ = 64 - vb         # partition base of sumexp rows
                    ctx = psC.tile([128, 512], f32, tag="ctx")
                    for m in range(MT):
                        sc = psS.tile([128, 512], f32, tag="sc")
                        nc.tensor.matmul(
                            sc[:],
                            kt_sb[vb:vb + 64, hb, m * 128:(m + 1) * 128],
                            qt_sb[vb:vb + 64, hb, n_sl],
                        )
                        spdt = spdp.tile([128, 512], mmdt, tag="spd")
                        nc.sync.dma_start(
                            spdt[:], spd[h, m * 128:(m + 1) * 128, n_sl]
                        )
                        s = work.tile([128, 512], f32, tag="s")
                        nc.vector.tensor_add(s[:], sc[:], spdt[:])
                        e = work.tile([128, 512], mmdt, tag="e")
                        nc.scalar.activation(e[:], s[:], Exp)
                        nc.tensor.matmul(
                            ctx[:],
                            v_sb[:, m, h * 128:(h + 1) * 128],
                            e[:],
                            start=(m == 0),
                            stop=(m == MT - 1),
                        )
                    # normalize: ctxT = ctx_vals * keep_scale / sumexp
                    # custom-DVE ops only behave at partition base 0 on HW:
                    # for even heads the sumexp row sits at partition 64, so
                    # ACT-copy it to SBUF and DMA it down to partition 0.
                    rr = work.tile([128, 512], f32, tag="rr")
                    if sb_ == 0:
                        recip_in = ctx[0:1, :]
                    else:
                        sscr = work.tile([128, 512], f32, tag="sscr")
                        nc.scalar.copy(sscr[sb_:sb_ + 1, :], ctx[sb_:sb_ + 1, :])
                        sm0 = work.tile([128, 512], f32, tag="sm0")
                        nc.sync.dma_start(sm0[0:1, :], sscr[sb_:sb_ + 1, :])
                        recip_in = sm0[0:1, :]
                    nc.vector.reciprocal_approx_fast(rr[0:1, :], recip_in)
                    # broadcast 1/sumexp across the 64 value partitions via a
                    # K=1 ones-matmul (PE can cross partitions; DVE cannot)
                    rb = psS.tile([128, 512], f32, tag="sc", name=f"rb{h}_{n0}")
                    nc.tensor.matmul(
                        rb[vb:vb + 64, :],
                        ones_sb[0:1, :],
                        rr[0:1, :],
                    )
                    rbs = work.tile([128, 512], f32, tag="rbs")
                    nc.scalar.copy(rbs[vb:vb + 64, :], rb[vb:vb + 64, :])
                    nc.vector.scalar_tensor_tensor(
                        out=ct_sb[vb:vb + 64, hb, n_sl],
                        in0=ctx[vb:vb + 64, :],
                        scalar=float(keep_scale),
                        in1=rbs[vb:vb + 64, :],
                        op0=mult,
                        op1=mult,
                    )

                # ---- Phase C for this n-block ----
                for nt in range(n0 * 4, (n0 + 1) * 4):
                    for d0 in range(NB):
                        po = psA.tile([128, 512], f32, tag="acc",
                                      name=f"po{nt}_{d0}")
                        for kk in range(HD // 128):
                            nc.tensor.matmul(
                                po[:],
                                ct_sb[:, kk, nt * 128:(nt + 1) * 128],
                                wo_sb[:, kk, d0 * 512:(d0 + 1) * 512],
                                start=(kk == 0),
                                stop=(kk == HD // 128 - 1),
                            )
                        ot = work.tile([128, 512], f32, tag="o")
                        nc.scalar.copy(ot[:], po[:])
                        nc.sync.dma_start(
                            out[nt * 128:(nt + 1) * 128,
                                d0 * 512:(d0 + 1) * 512],
                            ot[:],
                        )

    nc.finalize()
    return nc


def _get_nc(H_c: int, n_kept: int, mode: str):
    key = (H_c, n_kept, mode)
    if key not in _cache:
        _cache[key] = _build(H_c, HEADS / n_kept, mode)
    return _cache[key]


def _prep_inputs(x1, x2, spd, head_keep, Wq, Wkv, Wo, mode="bf16"):
    """Slice/transpose/pad host-side into per-core input maps."""
    import ml_dtypes

    ndt = np.float32 if mode in ("f32", "f32r") else ml_dtypes.bfloat16
    kept = [int(i) for i in np.nonzero(head_keep)[0]]
    n_kept = len(kept)
    H_c = (n_kept + 1) // 2
    if H_c % 2:
        H_c += 1
    groups = [kept[:H_c], kept[H_c:]]

    Wk_full, Wv_full = Wkv[:, :INNER], Wkv[:, INNER:]

    in_maps = []
    for b in range(B):
        xqT = np.ascontiguousarray(x1[b].T).reshape(KT, 128, N).astype(ndt)
        xkT = np.ascontiguousarray(x2[b].T).reshape(KT, 128, N).astype(ndt)
        for g in range(2):
            heads = groups[g]
            HD = H_c * DIM_HEAD
            wq_c = np.zeros((DIM, HD), np.float32)
            wk_c = np.zeros((DIM, HD), np.float32)
            wv_c = np.zeros((DIM, HD), np.float32)
            wo_c = np.zeros((HD, DIM), np.float32)
            spd_c = np.zeros((H_c, N, N), ndt)
            for i, h in enumerate(heads):
                sl = slice(i * DIM_HEAD, (i + 1) * DIM_HEAD)
                hs = slice(h * DIM_HEAD, (h + 1) * DIM_HEAD)
                wq_c[:, sl] = Wq[:, hs] * SCALE
                wk_c[:, sl] = Wk_full[:, hs]
                wv_c[:, sl] = Wv_full[:, hs]
                wo_c[sl, :] = Wo[hs, :]
                spd_c[i] = spd[b, h].T.astype(ndt)
            in_maps.append({
                "xq": xqT,
                "xk": xkT,
                "wq": np.ascontiguousarray(
                    wq_c.reshape(KT, 128, HD).transpose(1, 0, 2)).astype(ndt),
                "wk": np.ascontiguousarray(
                    wk_c.reshape(KT, 128, HD).transpose(1, 0, 2)).astype(ndt),
                "wv": np.ascontiguousarray(
                    wv_c.reshape(KT, 128, HD).transpose(1, 0, 2)).astype(ndt),
                "wo": np.ascontiguousarray(
                    wo_c.reshape(HD // 128, 128, DIM).transpose(1, 0, 2)).astype(ndt),
                "spd": spd_c,
            })
    return in_maps, n_kept, H_c


def _run(nc, in_maps, trace=False, tmpdir=None):
    from concourse.bass_utils import run_bass_kernel_spmd

    return run_bass_kernel_spmd(
        nc, in_maps, core_ids=list(range(NCORES)), trace=trace, tmpdir=tmpdir
    )


def kernel(x1, x2, spd, head_keep, Wq, Wkv, Wo, bo, _trace=False, _tmpdir=None):
    x1 = np.asarray(x1, np.float32)
    x2 = np.asarray(x2, np.float32)
    spd = np.asarray(spd, np.float32)
    head_keep = np.asarray(head_keep)
    n_kept = int(head_keep.astype(np.int64).sum())
    if n_kept == 0:
        # reference: 16/0 = inf, 0*inf = nan everywhere
        return np.full((B, N, DIM), np.nan, np.float32)

    mode = os.environ.get("KERNEL_DTYPE", "bf16")
    in_maps, n_kept, H_c = _prep_inputs(
        x1, x2, spd, head_keep, Wq, Wkv, Wo, mode)
    nc = _get_nc(H_c, n_kept, mode)
    res = _run(nc, in_maps, trace=_trace, tmpdir=_tmpdir)

    out = np.empty((B, N, DIM), np.float32)
    bo32 = np.asarray(bo, np.float32)
    for b in range(B):
        out[b] = res.results[2 * b]["out"] + res.results[2 * b + 1]["out"] + bo32
    kernel._last_results = res
    return out
